# revision 22
# baseline (speedup 1.0000x reference)
"""Axial (per-row) pair attention kernel for Trainium2, 8-core SPMD.

Contract: kernel(**inputs) takes the FULL unsharded inputs from
setup_inputs() and returns the FULL (2,128,128,256) float32 output.

Sharding: the (b, s1) row axis (2*128 = 256 independent attention rows) is
split evenly across 8 NeuronCores; each core runs the identical Bass program
on its 32-row slice. All per-core differences (activations, masks, rotary
tables) are carried in the input data, so no on-device partition logic is
needed.

Math notes (validated against the reference in fp32 numpy):
 - LayerNorm gamma/beta are folded into the QKV weights/bias on the host.
 - Rotary: out = q*cos + (R @ q)*sin on the first 32 channels, where R is the
   32x32 rotate-half permutation matrix, applied in the transposed (channel,
   token) layout via a small PE matmul.
 - Softmax: scores are tiny (|s*scale| < 1), so exp is computed without
   max-subtraction; the key mask enters as a -1e9 bias inside the fused
   ACT exp (exp -> exact 0), and the denominator comes from an extra
   all-ones column appended to V.

Implementation notes:
 - All matmul operands must start at partition 0 (nonzero matmul base
   partitions crash the exec unit on this stack), so q/k are repacked into
   (32, head, tok) tiles; heads g and g+4 live at the same partitions of the
   two e-chunk PSUM tiles, so the repack is 4 two-head copies per tensor.
 - LN statistics run in a prologue so the ScalarE activation table is loaded
   exactly twice (Sqrt once, Exp once) instead of thrashing per row.
"""

import numpy as np

import concourse.bass as bass
import concourse.mybir as mybir
import concourse.tile as tile
from concourse import bacc
from concourse.bass_utils import run_bass_kernel_spmd
from concourse.masks import make_identity

N_CORES = 8
B, S, D = 2, 128, 256
H, HD, ROT = 8, 32, 32
NROWS = B * S
RPC = NROWS // N_CORES  # rows per core = 32
SCALE = HD ** -0.5
LN_EPS = 1e-5
MASK_BIAS = -1e9

F32 = mybir.dt.float32
F16 = mybir.dt.float16  # matmul-input dtype (fp32 accumulate in PSUM)


def _build_bass() -> bass.Bass:
    nc = bacc.Bacc(None)

    x = nc.dram_tensor("x", [RPC, S, D], F32, kind="ExternalInput")
    cos_t = nc.dram_tensor("cos_t", [ROT, RPC, S], F16, kind="ExternalInput")
    sin_t = nc.dram_tensor("sin_t", [ROT, RPC, S], F16, kind="ExternalInput")
    maskb = nc.dram_tensor("maskb", [S, RPC], F32, kind="ExternalInput")
    wqkv = nc.dram_tensor("wqkv", [2, 128, 3 * D], F16, kind="ExternalInput")
    wout = nc.dram_tensor("wout", [2, 128, D], F16, kind="ExternalInput")
    rt = nc.dram_tensor("rt", [ROT, ROT], F16, kind="ExternalInput")
    y = nc.dram_tensor("y", [RPC, S, D], F32, kind="ExternalOutput")

    with tile.TileContext(nc) as tc:
        with (
            tc.tile_pool(name="consts", bufs=1) as consts,
            tc.tile_pool(name="xpool", bufs=RPC) as xpool,
            tc.tile_pool(name="lnpool", bufs=6) as lnpool,
            tc.tile_pool(name="tpool", bufs=4) as tpool,
            tc.tile_pool(name="qkpool", bufs=4) as qkpool,
            tc.tile_pool(name="vpool", bufs=4) as vpool,
            tc.tile_pool(name="epool", bufs=4) as epool,
            tc.tile_pool(name="apool", bufs=4) as apool,
            tc.tile_pool(name="ypool", bufs=4) as ypool,
            tc.tile_pool(name="ps_t", bufs=2, space="PSUM") as ps_t,
            tc.tile_pool(name="ps_qkv", bufs=2, space="PSUM") as ps_qkv,
            tc.tile_pool(name="ps_s", bufs=2, space="PSUM") as ps_s,
            tc.tile_pool(name="ps_o", bufs=2, space="PSUM") as ps_o,
        ):
            # ---- constants ----
            ident = consts.tile([128, 128], F16)
            make_identity(nc, ident)
            wqkv_sb = consts.tile([128, 2, 3 * D], F16)
            for c in range(2):
                nc.sync.dma_start(out=wqkv_sb[:, c, :], in_=wqkv[c])
            wout_sb = consts.tile([128, 2, D], F16)
            for c in range(2):
                nc.sync.dma_start(out=wout_sb[:, c, :], in_=wout[c])
            rt_sb = consts.tile([ROT, ROT], F16)
            nc.sync.dma_start(out=rt_sb, in_=rt[:])
            maskb_sb = consts.tile([S, RPC], F32)
            nc.sync.dma_start(out=maskb_sb, in_=maskb[:])
            eps_sb = consts.tile([128, 1], F32)
            nc.vector.memset(eps_sb, LN_EPS)
            cos_sb = consts.tile([ROT, RPC, S], F16)
            sin_sb = consts.tile([ROT, RPC, S], F16)
            nc.sync.dma_start(out=cos_sb, in_=cos_t[:])
            nc.sync.dma_start(out=sin_sb, in_=sin_t[:])

            # ---- prologue: loads + LN statistics for all rows ----
            # (keeps Sqrt/Exp activation-table loads to one each)
            mv_all = consts.tile([S, RPC, 2], F32)
            rstd_all = consts.tile([S, RPC], F32)
            x_tiles = []
            for r in range(RPC):
                x_sb = xpool.tile([S, D], F32)
                nc.sync.dma_start(out=x_sb, in_=x[r])
                x_tiles.append(x_sb)
                stats = lnpool.tile([S, 6], F32, tag="stats")
                nc.vector.bn_stats(out=stats, in_=x_sb)
                nc.vector.bn_aggr(out=mv_all[:, r, :], in_=stats)
            for r in range(RPC):
                nc.scalar.activation(
                    out=rstd_all[:, r:r + 1], in_=mv_all[:, r, 1:2],
                    func=mybir.ActivationFunctionType.Sqrt,
                    bias=eps_sb, scale=1.0,
                )
            nc.vector.reciprocal(out=rstd_all, in_=rstd_all)

            for r in range(RPC):
                # ---- LN apply -> fp16 ----
                xn_sb = lnpool.tile([S, D], F16, tag="xn")
                nc.vector.tensor_scalar(
                    out=xn_sb, in0=x_tiles[r],
                    scalar1=mv_all[:, r, 0:1], scalar2=rstd_all[:, r:r + 1],
                    op0=mybir.AluOpType.subtract, op1=mybir.AluOpType.mult,
                )

                # ---- transpose xn -> (d, tok) fp16, single wide copy ----
                t_ps = ps_t.tile([128, 2, S], F16, tag="tps")
                for c in range(2):
                    nc.tensor.transpose(
                        t_ps[:, c, :], xn_sb[:, c * 128:(c + 1) * 128], ident
                    )
                xnT_sb = tpool.tile([128, 2, S], F16)
                nc.scalar.copy(
                    out=xnT_sb.rearrange("p c s -> p (c s)"),
                    in_=t_ps.rearrange("p c s -> p (c s)"),
                )

                # ---- QKV projection ----
                q_ps = ps_qkv.tile([128, 2, S], F32, tag="qkv")
                k_ps = ps_qkv.tile([128, 2, S], F32, tag="qkv")
                for ec in range(2):
                    for dc in range(2):
                        nc.tensor.matmul(
                            q_ps[:, ec, :],
                            lhsT=wqkv_sb[:, dc, ec * 128:(ec + 1) * 128],
                            rhs=xnT_sb[:, dc, :],
                            start=(dc == 0), stop=(dc == 1),
                        )
                for ec in range(2):
                    for dc in range(2):
                        nc.tensor.matmul(
                            k_ps[:, ec, :],
                            lhsT=wqkv_sb[:, dc, D + ec * 128:D + (ec + 1) * 128],
                            rhs=xnT_sb[:, dc, :],
                            start=(dc == 0), stop=(dc == 1),
                        )
                v_ps = ps_qkv.tile([S, D], F32, tag="qkv")
                for dc in range(2):
                    nc.tensor.matmul(
                        v_ps,
                        lhsT=xnT_sb[:, dc, :],
                        rhs=wqkv_sb[:, dc, 2 * D:3 * D],
                        start=(dc == 0), stop=(dc == 1),
                    )

                # ---- repack q/k to (32, head, tok), base partition 0.
                # Heads g and g+4 sit at partitions 32g of the two e-chunks,
                # so one copy per partition-group moves two heads. ----
                qT_sb = qkpool.tile([HD, H, S], F16, tag="qT")
                kT_sb = qkpool.tile([HD, H, S], F16, tag="kT")
                qv = qT_sb.rearrange("p (b g) s -> p b g s", b=2)
                kv = kT_sb.rearrange("p (b g) s -> p b g s", b=2)
                for g in range(4):
                    nc.vector.tensor_copy(
                        out=qv[:, :, g, :], in_=q_ps[32 * g:32 * g + 32, :, :]
                    )
                    nc.scalar.copy(
                        out=kv[:, :, g, :], in_=k_ps[32 * g:32 * g + 32, :, :]
                    )
                # v with an extra all-ones column per head (softmax denom)
                v_sb = vpool.tile([S, H, HD + 1], F16)
                nc.vector.memset(v_sb[:, :, HD:HD + 1], 1.0)
                nc.vector.tensor_copy(
                    out=v_sb[:, :, 0:HD],
                    in_=v_ps.rearrange("p (h c) -> p h c", c=HD),
                )

                # ---- rotary on first 32 channels (head 0) of q and k ----
                cos_r = cos_sb[:, r, :]
                sin_r = sin_sb[:, r, :]
                rh_ps = ps_o.tile([ROT, 2, S], F32, tag="ops")
                for i, t_sb in enumerate((qT_sb, kT_sb)):
                    nc.tensor.matmul(
                        rh_ps[:, i, :], lhsT=rt_sb, rhs=t_sb[:, 0, :],
                        start=True, stop=True,
                    )
                for i, t_sb in enumerate((qT_sb, kT_sb)):
                    tmp_sb = qkpool.tile([ROT, S], F16, tag="rtmp")
                    nc.vector.tensor_mul(out=tmp_sb, in0=rh_ps[:, i, :], in1=sin_r)
                    nc.vector.tensor_mul(
                        out=t_sb[:, 0, :], in0=t_sb[:, 0, :], in1=cos_r
                    )
                    nc.vector.tensor_add(
                        out=t_sb[:, 0, :], in0=t_sb[:, 0, :], in1=tmp_sb
                    )

                # ---- scores^T and fused masked exp, in 4-head halves ----
                expT_sb = epool.tile([S, H, S], F16)
                for half in range(2):
                    s_ps = ps_s.tile([S, 4, S], F32, tag="sps")
                    for hh in range(4):
                        h = half * 4 + hh
                        nc.tensor.matmul(
                            s_ps[:, hh, :],
                            lhsT=kT_sb[:, h, :],
                            rhs=qT_sb[:, h, :],
                            start=True, stop=True,
                        )
                    nc.scalar.activation(
                        out=expT_sb[:, half * 4:(half + 1) * 4, :].rearrange(
                            "p h s -> p (h s)"
                        ),
                        in_=s_ps.rearrange("p h s -> p (h s)"),
                        func=mybir.ActivationFunctionType.Exp,
                        bias=maskb_sb[:, r:r + 1], scale=SCALE,
                    )

                # ---- attn @ [v | 1] ----
                o_ps = ps_o.tile([S, H, HD + 1], F32, tag="ops")
                for h in range(H):
                    nc.tensor.matmul(
                        o_ps[:, h, :],
                        lhsT=expT_sb[:, h, :],
                        rhs=v_sb[:, h, :],
                        start=True, stop=True,
                    )

                # ---- normalize via broadcast multiply -> (tok, d) fp16 ----
                recip = apool.tile([S, H], F32, tag="recip")
                nc.vector.reciprocal(out=recip, in_=o_ps[:, :, HD])
                attn_sb = apool.tile([S, H, HD], F16, tag="attn")
                recip_b = bass.AP(
                    tensor=recip.tensor, offset=recip.offset,
                    ap=list(recip.ap) + [[0, HD]],
                )
                nc.vector.tensor_mul(
                    out=attn_sb, in0=o_ps[:, :, 0:HD], in1=recip_b
                )

                # ---- transpose attn -> (d, tok); final projection ----
                t2_ps = ps_t.tile([128, 2, S], F16, tag="tps")
                attn_flat = attn_sb.rearrange("p h c -> p (h c)")
                for c in range(2):
                    nc.tensor.transpose(
                        t2_ps[:, c, :], attn_flat[:, c * 128:(c + 1) * 128], ident
                    )
                attnT_sb = apool.tile([128, 2, S], F16, tag="attnT")
                nc.scalar.copy(
                    out=attnT_sb.rearrange("p c s -> p (c s)"),
                    in_=t2_ps.rearrange("p c s -> p (c s)"),
                )

                y_ps = ps_o.tile([S, D], F32, tag="ops")
                for c in range(2):
                    nc.tensor.matmul(
                        y_ps,
                        lhsT=attnT_sb[:, c, :],
                        rhs=wout_sb[:, c, :],
                        start=(c == 0), stop=(c == 1),
                    )
                y_sb = ypool.tile([S, D], F32)
                nc.scalar.copy(out=y_sb, in_=y_ps)
                nc.sync.dma_start(out=y[r], in_=y_sb)

    nc.finalize()
    return nc


_NC = None


def _get_nc():
    global _NC
    if _NC is None:
        _NC = _build_bass()
    return _NC


def _host_prep(pair_act, pair_mask, ln_gamma, ln_beta, Wqkv, Wout):
    """Build the 8 per-core input maps (numpy only)."""
    pair_act = np.ascontiguousarray(pair_act, dtype=np.float32)
    ln_gamma = np.asarray(ln_gamma, dtype=np.float32)
    ln_beta = np.asarray(ln_beta, dtype=np.float32)
    Wqkv = np.asarray(Wqkv, dtype=np.float32)
    Wout = np.asarray(Wout, dtype=np.float32)

    # fold gamma/beta into the QKV projection (beta term is exactly zero for
    # the reference's beta=0, and the kernel does not apply a qkv bias;
    # assert so a nonzero-beta input cannot silently produce wrong results)
    W_eff = (Wqkv * ln_gamma[None, :]).T  # (256, 768): qkv = xn_z @ W_eff
    bias_eff = ln_beta @ Wqkv.T
    assert np.abs(bias_eff).max() == 0.0, "nonzero LN beta not supported"

    wqkv_h = W_eff.reshape(2, 128, 3 * D).astype(np.float16)
    wout_h = Wout.T.reshape(2, 128, D).astype(np.float16)

    # rotary tables (transposed): table[s1, c, y]
    inv_freq = 1.0 / (10000.0 ** (np.arange(0, 16, dtype=np.float32)[::2] / 16.0))
    t = np.linspace(-1.0, 1.0, S, dtype=np.float32)
    f = np.repeat(t[:, None] * inv_freq[None, :], 2, axis=-1)  # (S, 16)
    cosT = np.empty((S, ROT, S), np.float32)
    sinT = np.empty((S, ROT, S), np.float32)
    cosT[:, :16, :] = np.cos(f)[:, :, None]
    sinT[:, :16, :] = np.sin(f)[:, :, None]
    cosT[:, 16:, :] = np.cos(f).T[None, :, :]
    sinT[:, 16:, :] = np.sin(f).T[None, :, :]
    cosT = cosT.astype(np.float16)
    sinT = sinT.astype(np.float16)

    # rotate-half matrix, transposed for the PE (lhsT)
    R = np.zeros((ROT, ROT), np.float32)
    for j in range(ROT // 2):
        R[2 * j, 2 * j + 1] = -1.0
        R[2 * j + 1, 2 * j] = 1.0
    rt_h = R.T.astype(np.float16)

    x_all = pair_act.reshape(NROWS, S, D)
    maskb_all = np.where(
        np.asarray(pair_mask, bool), np.float32(MASK_BIAS), np.float32(0.0)
    ).reshape(NROWS, S)

    in_maps = []
    for core in range(N_CORES):
        r0 = core * RPC
        rows = slice(r0, r0 + RPC)
        s1 = np.arange(r0, r0 + RPC) % S
        in_maps.append({
            "x": x_all[rows],
            "cos_t": np.ascontiguousarray(cosT[s1].transpose(1, 0, 2)),
            "sin_t": np.ascontiguousarray(sinT[s1].transpose(1, 0, 2)),
            "maskb": np.ascontiguousarray(maskb_all[rows].T),  # (S, RPC)
            "wqkv": wqkv_h,
            "wout": wout_h,
            "rt": rt_h,
        })
    return in_maps


def kernel(pair_act, pair_mask, ln_gamma, ln_beta, Wqkv, Wout):
    in_maps = _host_prep(pair_act, pair_mask, ln_gamma, ln_beta, Wqkv, Wout)
    nc = _get_nc()
    res = run_bass_kernel_spmd(nc, in_maps, core_ids=list(range(N_CORES)))
    y = np.stack([res.results[i]["y"] for i in range(N_CORES)])
    return y.reshape(B, S, S, D).astype(np.float32)


# revision 26
# speedup vs baseline: 1.4783x; 1.4783x over previous
"""Axial (per-row) pair attention kernel for Trainium2, 8-core SPMD.

Contract: kernel(**inputs) takes the FULL unsharded inputs from
setup_inputs() and returns the FULL (2,128,128,256) float32 output.

Sharding: the (b, s1) row axis (2*128 = 256 independent attention rows) is
split evenly across 8 NeuronCores; each core runs the identical Bass program
on its 32-row slice. All per-core differences (activations, masks, rotary
tables) are carried in the input data, so no on-device partition logic is
needed.

Math notes (validated against the reference in fp32 numpy):
 - LayerNorm gamma/beta are folded into the QKV weights/bias on the host.
 - Rotary: out = q*cos + (R @ q)*sin on the first 32 channels, where R is the
   32x32 rotate-half permutation matrix, applied in the transposed (channel,
   token) layout via a small PE matmul.
 - Softmax: scores are tiny (|s*scale| < 1), so exp is computed without
   max-subtraction; the key mask enters as a -1e9 bias inside the fused
   ACT exp (exp -> exact 0), and the denominator comes from an extra
   all-ones column appended to V.

Implementation notes:
 - All matmul operands must start at partition 0 (nonzero matmul base
   partitions crash the exec unit on this stack), so q/k are repacked into
   (32, head, tok) tiles; heads g and g+4 live at the same partitions of the
   two e-chunk PSUM tiles, so the repack is 4 two-head copies per tensor.
 - LN statistics run in a prologue so the ScalarE activation table is loaded
   exactly twice (Sqrt once, Exp once) instead of thrashing per row.
"""

import numpy as np

import concourse.bass as bass
import concourse.mybir as mybir
import concourse.tile as tile
from concourse import bacc
from concourse.bass_utils import run_bass_kernel_spmd
from concourse.masks import make_identity

N_CORES = 8
B, S, D = 2, 128, 256
H, HD, ROT = 8, 32, 32
NROWS = B * S
RPC = NROWS // N_CORES  # rows per core = 32
SCALE = HD ** -0.5
LN_EPS = 1e-5
MASK_BIAS = -1e9

F32 = mybir.dt.float32
F16 = mybir.dt.float16  # matmul-input dtype (fp32 accumulate in PSUM)


def _build_bass() -> bass.Bass:
    nc = bacc.Bacc(None)

    x = nc.dram_tensor("x", [RPC, S, D], F32, kind="ExternalInput")
    cos_t = nc.dram_tensor("cos_t", [ROT, RPC, S], F16, kind="ExternalInput")
    sin_t = nc.dram_tensor("sin_t", [ROT, RPC, S], F16, kind="ExternalInput")
    maskb = nc.dram_tensor("maskb", [S, RPC], F32, kind="ExternalInput")
    wqkv = nc.dram_tensor("wqkv", [2, 128, 3 * D], F16, kind="ExternalInput")
    wout = nc.dram_tensor("wout", [2, 128, D], F16, kind="ExternalInput")
    rt = nc.dram_tensor("rt", [ROT, ROT], F16, kind="ExternalInput")
    y = nc.dram_tensor("y", [RPC, S, D], F32, kind="ExternalOutput")

    with tile.TileContext(nc) as tc:
        with (
            tc.tile_pool(name="consts", bufs=1) as consts,
            tc.tile_pool(name="xpool", bufs=RPC) as xpool,
            tc.tile_pool(name="lnpool", bufs=6) as lnpool,
            tc.tile_pool(name="tpool", bufs=4) as tpool,
            tc.tile_pool(name="qkpool", bufs=4) as qkpool,
            tc.tile_pool(name="vpool", bufs=4) as vpool,
            tc.tile_pool(name="epool", bufs=4) as epool,
            tc.tile_pool(name="apool", bufs=4) as apool,
            tc.tile_pool(name="ypool", bufs=4) as ypool,
            tc.tile_pool(name="ps_t", bufs=2, space="PSUM") as ps_t,
            tc.tile_pool(name="ps_qkv", bufs=2, space="PSUM") as ps_qkv,
            tc.tile_pool(name="ps_s", bufs=2, space="PSUM") as ps_s,
            tc.tile_pool(name="ps_o", bufs=2, space="PSUM") as ps_o,
        ):
            # ---- constants ----
            ident = consts.tile([128, 128], F16)
            make_identity(nc, ident)
            wqkv_sb = consts.tile([128, 2, 3 * D], F16)
            for c in range(2):
                nc.sync.dma_start(out=wqkv_sb[:, c, :], in_=wqkv[c])
            wout_sb = consts.tile([128, 2, D], F16)
            for c in range(2):
                nc.sync.dma_start(out=wout_sb[:, c, :], in_=wout[c])
            rt_sb = consts.tile([ROT, ROT], F16)
            nc.sync.dma_start(out=rt_sb, in_=rt[:])
            maskb_sb = consts.tile([S, RPC], F32)
            nc.sync.dma_start(out=maskb_sb, in_=maskb[:])
            eps_sb = consts.tile([128, 1], F32)
            nc.vector.memset(eps_sb, LN_EPS)
            cos_sb = consts.tile([ROT, RPC, S], F16)
            sin_sb = consts.tile([ROT, RPC, S], F16)
            nc.sync.dma_start(out=cos_sb, in_=cos_t[:])
            nc.sync.dma_start(out=sin_sb, in_=sin_t[:])

            # ---- prologue: loads + LN statistics for all rows ----
            # (keeps Sqrt/Exp activation-table loads to one each)
            mv_all = consts.tile([S, RPC, 2], F32)
            rstd_all = consts.tile([S, RPC], F32)
            x_tiles = []
            for r in range(RPC):
                x_sb = xpool.tile([S, D], F32)
                nc.sync.dma_start(out=x_sb, in_=x[r])
                x_tiles.append(x_sb)
                stats = lnpool.tile([S, 6], F32, tag="stats")
                nc.vector.bn_stats(out=stats, in_=x_sb)
                nc.vector.bn_aggr(out=mv_all[:, r, :], in_=stats)
            for r in range(RPC):
                nc.scalar.activation(
                    out=rstd_all[:, r:r + 1], in_=mv_all[:, r, 1:2],
                    func=mybir.ActivationFunctionType.Sqrt,
                    bias=eps_sb, scale=1.0,
                )
                nc.vector.reciprocal(
                    out=rstd_all[:, r:r + 1], in_=rstd_all[:, r:r + 1]
                )

            def phase1(r):
                # LN apply, transpose, QKV, repack, v, rotary
                xn_sb = lnpool.tile([S, D], F16, tag="xn")
                nc.vector.tensor_scalar(
                    out=xn_sb, in0=x_tiles[r],
                    scalar1=mv_all[:, r, 0:1], scalar2=rstd_all[:, r:r + 1],
                    op0=mybir.AluOpType.subtract, op1=mybir.AluOpType.mult,
                )

                # ---- transpose xn -> (d, tok) fp16, single wide copy ----
                t_ps = ps_t.tile([128, 2, S], F16, tag="tps")
                for c in range(2):
                    nc.tensor.transpose(
                        t_ps[:, c, :], xn_sb[:, c * 128:(c + 1) * 128], ident
                    )
                xnT_sb = tpool.tile([128, 2, S], F16)
                nc.scalar.copy(
                    out=xnT_sb.rearrange("p c s -> p (c s)"),
                    in_=t_ps.rearrange("p c s -> p (c s)"),
                )

                # ---- QKV projection ----
                q_ps = ps_qkv.tile([128, 2, S], F32, tag="qkv")
                k_ps = ps_qkv.tile([128, 2, S], F32, tag="qkv")
                for ec in range(2):
                    for dc in range(2):
                        nc.tensor.matmul(
                            q_ps[:, ec, :],
                            lhsT=wqkv_sb[:, dc, ec * 128:(ec + 1) * 128],
                            rhs=xnT_sb[:, dc, :],
                            start=(dc == 0), stop=(dc == 1),
                        )
                for ec in range(2):
                    for dc in range(2):
                        nc.tensor.matmul(
                            k_ps[:, ec, :],
                            lhsT=wqkv_sb[:, dc, D + ec * 128:D + (ec + 1) * 128],
                            rhs=xnT_sb[:, dc, :],
                            start=(dc == 0), stop=(dc == 1),
                        )
                v_ps = ps_qkv.tile([S, D], F32, tag="qkv")
                for dc in range(2):
                    nc.tensor.matmul(
                        v_ps,
                        lhsT=xnT_sb[:, dc, :],
                        rhs=wqkv_sb[:, dc, 2 * D:3 * D],
                        start=(dc == 0), stop=(dc == 1),
                    )

                # ---- repack q/k to (32, head, tok), base partition 0.
                # Heads g and g+4 sit at partitions 32g of the two e-chunks,
                # so one copy per partition-group moves two heads. ----
                qT_sb = qkpool.tile([HD, H, S], F16, tag="qT")
                kT_sb = qkpool.tile([HD, H, S], F16, tag="kT")
                qv = qT_sb.rearrange("p (b g) s -> p b g s", b=2)
                kv = kT_sb.rearrange("p (b g) s -> p b g s", b=2)
                for g in range(4):
                    nc.vector.tensor_copy(
                        out=qv[:, :, g, :], in_=q_ps[32 * g:32 * g + 32, :, :]
                    )
                    nc.scalar.copy(
                        out=kv[:, :, g, :], in_=k_ps[32 * g:32 * g + 32, :, :]
                    )
                # v with an extra all-ones column per head (softmax denom)
                v_sb = vpool.tile([S, H, HD + 1], F16)
                nc.vector.memset(v_sb[:, :, HD:HD + 1], 1.0)
                nc.vector.tensor_copy(
                    out=v_sb[:, :, 0:HD],
                    in_=v_ps.rearrange("p (h c) -> p h c", c=HD),
                )

                # ---- rotary on first 32 channels (head 0) of q and k ----
                cos_r = cos_sb[:, r, :]
                sin_r = sin_sb[:, r, :]
                rh_ps = ps_o.tile([ROT, 2, S], F32, tag="ops")
                for i, t_sb in enumerate((qT_sb, kT_sb)):
                    nc.tensor.matmul(
                        rh_ps[:, i, :], lhsT=rt_sb, rhs=t_sb[:, 0, :],
                        start=True, stop=True,
                    )
                for i, t_sb in enumerate((qT_sb, kT_sb)):
                    tmp_sb = qkpool.tile([ROT, S], F16, tag="rtmp")
                    nc.vector.tensor_mul(out=tmp_sb, in0=rh_ps[:, i, :], in1=sin_r)
                    nc.vector.tensor_mul(
                        out=t_sb[:, 0, :], in0=t_sb[:, 0, :], in1=cos_r
                    )
                    nc.vector.tensor_add(
                        out=t_sb[:, 0, :], in0=t_sb[:, 0, :], in1=tmp_sb
                    )
                return {"qT": qT_sb, "kT": kT_sb, "v": v_sb}

            def phase2(r, st):
                # scores + exp + attn@[v|1] + normalize
                qT_sb, kT_sb, v_sb = st["qT"], st["kT"], st["v"]
                # ---- scores^T and fused masked exp, in 4-head halves ----
                expT_sb = epool.tile([S, H, S], F16)
                for half in range(2):
                    s_ps = ps_s.tile([S, 4, S], F32, tag="sps")
                    for hh in range(4):
                        h = half * 4 + hh
                        nc.tensor.matmul(
                            s_ps[:, hh, :],
                            lhsT=kT_sb[:, h, :],
                            rhs=qT_sb[:, h, :],
                            start=True, stop=True,
                        )
                    nc.scalar.activation(
                        out=expT_sb[:, half * 4:(half + 1) * 4, :].rearrange(
                            "p h s -> p (h s)"
                        ),
                        in_=s_ps.rearrange("p h s -> p (h s)"),
                        func=mybir.ActivationFunctionType.Exp,
                        bias=maskb_sb[:, r:r + 1], scale=SCALE,
                    )

                # ---- attn @ [v | 1] ----
                o_ps = ps_o.tile([S, H, HD + 1], F32, tag="ops")
                for h in range(H):
                    nc.tensor.matmul(
                        o_ps[:, h, :],
                        lhsT=expT_sb[:, h, :],
                        rhs=v_sb[:, h, :],
                        start=True, stop=True,
                    )

                # ---- normalize via broadcast multiply -> (tok, d) fp16 ----
                recip = apool.tile([S, H], F32, tag="recip")
                nc.vector.reciprocal(out=recip, in_=o_ps[:, :, HD])
                attn_sb = apool.tile([S, H, HD], F16, tag="attn")
                recip_b = bass.AP(
                    tensor=recip.tensor, offset=recip.offset,
                    ap=list(recip.ap) + [[0, HD]],
                )
                nc.vector.tensor_mul(
                    out=attn_sb, in0=o_ps[:, :, 0:HD], in1=recip_b
                )
                st["attn"] = attn_sb

            def phase3(r, st):
                attn_sb = st["attn"]
                # ---- transpose attn -> (d, tok); final projection ----
                t2_ps = ps_t.tile([128, 2, S], F16, tag="tps")
                attn_flat = attn_sb.rearrange("p h c -> p (h c)")
                for c in range(2):
                    nc.tensor.transpose(
                        t2_ps[:, c, :], attn_flat[:, c * 128:(c + 1) * 128], ident
                    )
                attnT_sb = apool.tile([128, 2, S], F16, tag="attnT")
                nc.scalar.copy(
                    out=attnT_sb.rearrange("p c s -> p (c s)"),
                    in_=t2_ps.rearrange("p c s -> p (c s)"),
                )

                y_ps = ps_o.tile([S, D], F32, tag="ops")
                for c in range(2):
                    nc.tensor.matmul(
                        y_ps,
                        lhsT=attnT_sb[:, c, :],
                        rhs=wout_sb[:, c, :],
                        start=(c == 0), stop=(c == 1),
                    )
                y_sb = ypool.tile([S, D], F32)
                nc.scalar.copy(out=y_sb, in_=y_ps)
                nc.sync.dma_start(out=y[r], in_=y_sb)

            # software-pipelined skew: rows i / i-1 / i-2 in flight so each
            # engine's static order interleaves adjacent rows' work
            state = {}
            for i in range(RPC + 2):
                if i < RPC:
                    state[i] = phase1(i)
                if 0 <= i - 1 < RPC:
                    phase2(i - 1, state[i - 1])
                if 0 <= i - 2 < RPC:
                    phase3(i - 2, state.pop(i - 2))

    nc.finalize()
    return nc


_NC = None


def _get_nc():
    global _NC
    if _NC is None:
        _NC = _build_bass()
    return _NC


def _host_prep(pair_act, pair_mask, ln_gamma, ln_beta, Wqkv, Wout):
    """Build the 8 per-core input maps (numpy only)."""
    pair_act = np.ascontiguousarray(pair_act, dtype=np.float32)
    ln_gamma = np.asarray(ln_gamma, dtype=np.float32)
    ln_beta = np.asarray(ln_beta, dtype=np.float32)
    Wqkv = np.asarray(Wqkv, dtype=np.float32)
    Wout = np.asarray(Wout, dtype=np.float32)

    # fold gamma/beta into the QKV projection (beta term is exactly zero for
    # the reference's beta=0, and the kernel does not apply a qkv bias;
    # assert so a nonzero-beta input cannot silently produce wrong results)
    W_eff = (Wqkv * ln_gamma[None, :]).T  # (256, 768): qkv = xn_z @ W_eff
    bias_eff = ln_beta @ Wqkv.T
    assert np.abs(bias_eff).max() == 0.0, "nonzero LN beta not supported"

    wqkv_h = W_eff.reshape(2, 128, 3 * D).astype(np.float16)
    wout_h = Wout.T.reshape(2, 128, D).astype(np.float16)

    # rotary tables (transposed): table[s1, c, y]
    inv_freq = 1.0 / (10000.0 ** (np.arange(0, 16, dtype=np.float32)[::2] / 16.0))
    t = np.linspace(-1.0, 1.0, S, dtype=np.float32)
    f = np.repeat(t[:, None] * inv_freq[None, :], 2, axis=-1)  # (S, 16)
    cosT = np.empty((S, ROT, S), np.float32)
    sinT = np.empty((S, ROT, S), np.float32)
    cosT[:, :16, :] = np.cos(f)[:, :, None]
    sinT[:, :16, :] = np.sin(f)[:, :, None]
    cosT[:, 16:, :] = np.cos(f).T[None, :, :]
    sinT[:, 16:, :] = np.sin(f).T[None, :, :]
    cosT = cosT.astype(np.float16)
    sinT = sinT.astype(np.float16)

    # rotate-half matrix, transposed for the PE (lhsT)
    R = np.zeros((ROT, ROT), np.float32)
    for j in range(ROT // 2):
        R[2 * j, 2 * j + 1] = -1.0
        R[2 * j + 1, 2 * j] = 1.0
    rt_h = R.T.astype(np.float16)

    x_all = pair_act.reshape(NROWS, S, D)
    maskb_all = np.where(
        np.asarray(pair_mask, bool), np.float32(MASK_BIAS), np.float32(0.0)
    ).reshape(NROWS, S)

    in_maps = []
    for core in range(N_CORES):
        r0 = core * RPC
        rows = slice(r0, r0 + RPC)
        s1 = np.arange(r0, r0 + RPC) % S
        in_maps.append({
            "x": x_all[rows],
            "cos_t": np.ascontiguousarray(cosT[s1].transpose(1, 0, 2)),
            "sin_t": np.ascontiguousarray(sinT[s1].transpose(1, 0, 2)),
            "maskb": np.ascontiguousarray(maskb_all[rows].T),  # (S, RPC)
            "wqkv": wqkv_h,
            "wout": wout_h,
            "rt": rt_h,
        })
    return in_maps


def kernel(pair_act, pair_mask, ln_gamma, ln_beta, Wqkv, Wout):
    in_maps = _host_prep(pair_act, pair_mask, ln_gamma, ln_beta, Wqkv, Wout)
    nc = _get_nc()
    res = run_bass_kernel_spmd(nc, in_maps, core_ids=list(range(N_CORES)))
    y = np.stack([res.results[i]["y"] for i in range(N_CORES)])
    return y.reshape(B, S, S, D).astype(np.float32)


# revision 29
# speedup vs baseline: 1.6760x; 1.1337x over previous
"""Axial (per-row) pair attention kernel for Trainium2, 8-core SPMD.

Contract: kernel(**inputs) takes the FULL unsharded inputs from
setup_inputs() and returns the FULL (2,128,128,256) float32 output.

Sharding: the (b, s1) row axis (2*128 = 256 independent attention rows) is
split evenly across 8 NeuronCores; each core runs the identical Bass program
on its 32-row slice. All per-core differences (activations, masks, rotary
tables) are carried in the input data, so no on-device partition logic is
needed.

Math notes (validated against the reference in fp32 numpy):
 - LayerNorm gamma/beta are folded into the QKV weights/bias on the host.
 - Rotary: out = q*cos + (R @ q)*sin on the first 32 channels, where R is the
   32x32 rotate-half permutation matrix, applied in the transposed (channel,
   token) layout via a small PE matmul.
 - Softmax: scores are tiny (|s*scale| < 1), so exp is computed without
   max-subtraction; the key mask enters as a -1e9 bias inside the fused
   ACT exp (exp -> exact 0), and the denominator comes from an extra
   all-ones column appended to V.

Implementation notes:
 - All matmul operands must start at partition 0 (nonzero matmul base
   partitions crash the exec unit on this stack), so q/k are repacked into
   (32, head, tok) tiles; heads g and g+4 live at the same partitions of the
   two e-chunk PSUM tiles, so the repack is 4 two-head copies per tensor.
 - LN statistics run in a prologue so the ScalarE activation table is loaded
   exactly twice (Sqrt once, Exp once) instead of thrashing per row.
"""

import numpy as np

import concourse.bass as bass
import concourse.mybir as mybir
import concourse.tile as tile
from concourse import bacc
from concourse.bass_utils import run_bass_kernel_spmd
from concourse.masks import make_identity

N_CORES = 8
B, S, D = 2, 128, 256
H, HD, ROT = 8, 32, 32
NROWS = B * S
RPC = NROWS // N_CORES  # rows per core = 32
SCALE = HD ** -0.5
LN_EPS = 1e-5
MASK_BIAS = -1e9

F32 = mybir.dt.float32
F16 = mybir.dt.float16  # matmul-input dtype (fp32 accumulate in PSUM)


def _build_bass() -> bass.Bass:
    nc = bacc.Bacc(None)

    x = nc.dram_tensor("x", [RPC, S, D], F32, kind="ExternalInput")
    cos_t = nc.dram_tensor("cos_t", [ROT, RPC, S], F16, kind="ExternalInput")
    sin_t = nc.dram_tensor("sin_t", [ROT, RPC, S], F16, kind="ExternalInput")
    maskb = nc.dram_tensor("maskb", [S, RPC], F32, kind="ExternalInput")
    wqkv = nc.dram_tensor("wqkv", [2, 128, 3 * D], F16, kind="ExternalInput")
    wout = nc.dram_tensor("wout", [2, 128, D], F16, kind="ExternalInput")
    rt = nc.dram_tensor("rt", [ROT, ROT], F16, kind="ExternalInput")
    y = nc.dram_tensor("y", [RPC, S, D], F32, kind="ExternalOutput")

    with tile.TileContext(nc) as tc:
        with (
            tc.tile_pool(name="consts", bufs=1) as consts,
            tc.tile_pool(name="xpool", bufs=RPC) as xpool,
            tc.tile_pool(name="lnpool", bufs=6) as lnpool,
            tc.tile_pool(name="tpool", bufs=4) as tpool,
            tc.tile_pool(name="qkpool", bufs=4) as qkpool,
            tc.tile_pool(name="vpool", bufs=4) as vpool,
            tc.tile_pool(name="epool", bufs=4) as epool,
            tc.tile_pool(name="apool", bufs=4) as apool,
            tc.tile_pool(name="ypool", bufs=4) as ypool,
            tc.tile_pool(name="ps_t", bufs=2, space="PSUM") as ps_t,
            tc.tile_pool(name="ps_qkv", bufs=2, space="PSUM") as ps_qkv,
            tc.tile_pool(name="ps_s", bufs=2, space="PSUM") as ps_s,
            tc.tile_pool(name="ps_o", bufs=2, space="PSUM") as ps_o,
        ):
            # ---- constants ----
            ident = consts.tile([128, 128], F16)
            make_identity(nc, ident)
            wqkv_sb = consts.tile([128, 2, 3 * D], F16)
            for c in range(2):
                nc.sync.dma_start(out=wqkv_sb[:, c, :], in_=wqkv[c])
            wout_sb = consts.tile([128, 2, D], F16)
            for c in range(2):
                nc.sync.dma_start(out=wout_sb[:, c, :], in_=wout[c])
            rt_sb = consts.tile([ROT, ROT], F16)
            nc.sync.dma_start(out=rt_sb, in_=rt[:])
            maskb_sb = consts.tile([S, RPC], F32)
            nc.sync.dma_start(out=maskb_sb, in_=maskb[:])
            eps_sb = consts.tile([128, 1], F32)
            nc.vector.memset(eps_sb, LN_EPS)
            cos_sb = consts.tile([ROT, RPC, S], F16)
            sin_sb = consts.tile([ROT, RPC, S], F16)
            nc.sync.dma_start(out=cos_sb, in_=cos_t[:])
            nc.sync.dma_start(out=sin_sb, in_=sin_t[:])

            # ---- prologue: loads + LN statistics for all rows ----
            # (keeps Sqrt/Exp activation-table loads to one each)
            mv_all = consts.tile([S, RPC, 2], F32)
            rstd_all = consts.tile([S, RPC], F32)
            x_tiles = []
            for r in range(RPC):
                x_sb = xpool.tile([S, D], F32)
                nc.sync.dma_start(out=x_sb, in_=x[r])
                x_tiles.append(x_sb)
                stats = lnpool.tile([S, 6], F32, tag="stats")
                nc.vector.bn_stats(out=stats, in_=x_sb)
                nc.vector.bn_aggr(out=mv_all[:, r, :], in_=stats)
            for r in range(RPC):
                nc.scalar.activation(
                    out=rstd_all[:, r:r + 1], in_=mv_all[:, r, 1:2],
                    func=mybir.ActivationFunctionType.Sqrt,
                    bias=eps_sb, scale=1.0,
                )
                nc.vector.reciprocal(
                    out=rstd_all[:, r:r + 1], in_=rstd_all[:, r:r + 1]
                )

            def phase1(r):
                # LN apply, transpose, QKV, repack, v, rotary
                xn_sb = lnpool.tile([S, D], F16, tag="xn")
                nc.vector.tensor_scalar(
                    out=xn_sb, in0=x_tiles[r],
                    scalar1=mv_all[:, r, 0:1], scalar2=rstd_all[:, r:r + 1],
                    op0=mybir.AluOpType.subtract, op1=mybir.AluOpType.mult,
                )

                # ---- transpose xn -> (d, tok) fp16, single wide copy ----
                t_ps = ps_t.tile([128, 2, S], F16, tag="tps")
                for c in range(2):
                    nc.tensor.transpose(
                        t_ps[:, c, :], xn_sb[:, c * 128:(c + 1) * 128], ident
                    )
                xnT_sb = tpool.tile([128, 2, S], F16)
                nc.scalar.copy(
                    out=xnT_sb.rearrange("p c s -> p (c s)"),
                    in_=t_ps.rearrange("p c s -> p (c s)"),
                )

                # ---- QKV projection; q and k share one PSUM bank so the
                # head repack moves q+k chunks in single copies ----
                qk_ps = ps_qkv.tile([128, 4, S], F32, tag="qkv")
                for qk in range(2):
                    for ec in range(2):
                        for dc in range(2):
                            nc.tensor.matmul(
                                qk_ps[:, qk * 2 + ec, :],
                                lhsT=wqkv_sb[
                                    :, dc, qk * D + ec * 128:qk * D + (ec + 1) * 128
                                ],
                                rhs=xnT_sb[:, dc, :],
                                start=(dc == 0), stop=(dc == 1),
                            )
                v_ps = ps_qkv.tile([S, D], F32, tag="qkv")
                for dc in range(2):
                    nc.tensor.matmul(
                        v_ps,
                        lhsT=xnT_sb[:, dc, :],
                        rhs=wqkv_sb[:, dc, 2 * D:3 * D],
                        start=(dc == 0), stop=(dc == 1),
                    )

                # ---- repack q/k to (32, qk, b, g, tok): head h = 4b+g of
                # q/k is [:, qk, b, g, :], base partition 0 always.
                # Partition-group g of both q and k moves in ONE copy. ----
                qkT_sb = qkpool.tile([HD, 2, 2, 4, S], F16, tag="qkT")
                for g in range(4):
                    eng = nc.vector if g % 2 == 0 else nc.scalar
                    (eng.tensor_copy if g % 2 == 0 else eng.copy)(
                        out=qkT_sb[:, :, :, g, :],
                        in_=qk_ps[32 * g:32 * g + 32, :, :],
                    )
                # v with an extra all-ones column per head (softmax denom)
                v_sb = vpool.tile([S, H, HD + 1], F16)
                nc.vector.memset(v_sb[:, :, HD:HD + 1], 1.0)
                nc.vector.tensor_copy(
                    out=v_sb[:, :, 0:HD],
                    in_=v_ps.rearrange("p (h c) -> p h c", c=HD),
                )

                # ---- rotary on first 32 channels (head 0) of q and k,
                # processed as one (32, 2, S) strided pair ----
                pv = qkT_sb[:, :, 0, 0, :]  # (32, qk, S)
                cos_r = cos_sb[:, r, :]
                sin_r = sin_sb[:, r, :]
                cos_b = bass.AP(
                    tensor=cos_sb.tensor, offset=cos_r.offset,
                    ap=[cos_r.ap[0], [0, 2], cos_r.ap[1]],
                )
                sin_b = bass.AP(
                    tensor=sin_sb.tensor, offset=sin_r.offset,
                    ap=[sin_r.ap[0], [0, 2], sin_r.ap[1]],
                )
                rh_ps = ps_o.tile([ROT, 2, S], F32, tag="ops")
                nc.tensor.matmul(
                    rh_ps, lhsT=rt_sb, rhs=pv, start=True, stop=True,
                )
                tmp_sb = qkpool.tile([ROT, 2, S], F16, tag="rtmp")
                nc.vector.tensor_mul(out=tmp_sb, in0=rh_ps, in1=sin_b)
                nc.vector.tensor_mul(out=pv, in0=pv, in1=cos_b)
                nc.vector.tensor_add(out=pv, in0=pv, in1=tmp_sb)
                return {"qkT": qkT_sb, "v": v_sb}

            def phase2(r, st):
                # scores + exp + attn@[v|1] + normalize
                qkT_sb, v_sb = st["qkT"], st["v"]
                # ---- scores^T and fused masked exp, in 4-head halves ----
                expT_sb = epool.tile([S, H, S], F16)
                for half in range(2):
                    s_ps = ps_s.tile([S, 4, S], F32, tag="sps")
                    for hh in range(4):
                        h = half * 4 + hh
                        nc.tensor.matmul(
                            s_ps[:, hh, :],
                            lhsT=qkT_sb[:, 1, h // 4, h % 4, :],
                            rhs=qkT_sb[:, 0, h // 4, h % 4, :],
                            start=True, stop=True,
                        )
                    nc.scalar.activation(
                        out=expT_sb[:, half * 4:(half + 1) * 4, :].rearrange(
                            "p h s -> p (h s)"
                        ),
                        in_=s_ps.rearrange("p h s -> p (h s)"),
                        func=mybir.ActivationFunctionType.Exp,
                        bias=maskb_sb[:, r:r + 1], scale=SCALE,
                    )

                # ---- attn @ [v | 1] ----
                o_ps = ps_o.tile([S, H, HD + 1], F32, tag="ops")
                for h in range(H):
                    nc.tensor.matmul(
                        o_ps[:, h, :],
                        lhsT=expT_sb[:, h, :],
                        rhs=v_sb[:, h, :],
                        start=True, stop=True,
                    )

                # ---- normalize via broadcast multiply -> (tok, d) fp16 ----
                recip = apool.tile([S, H], F32, tag="recip")
                nc.vector.reciprocal(out=recip, in_=o_ps[:, :, HD])
                attn_sb = apool.tile([S, H, HD], F16, tag="attn")
                recip_b = bass.AP(
                    tensor=recip.tensor, offset=recip.offset,
                    ap=list(recip.ap) + [[0, HD]],
                )
                nc.vector.tensor_mul(
                    out=attn_sb, in0=o_ps[:, :, 0:HD], in1=recip_b
                )
                st["attn"] = attn_sb

            def phase3(r, st):
                attn_sb = st["attn"]
                # ---- transpose attn -> (d, tok); final projection ----
                t2_ps = ps_t.tile([128, 2, S], F16, tag="tps")
                attn_flat = attn_sb.rearrange("p h c -> p (h c)")
                for c in range(2):
                    nc.tensor.transpose(
                        t2_ps[:, c, :], attn_flat[:, c * 128:(c + 1) * 128], ident
                    )
                attnT_sb = apool.tile([128, 2, S], F16, tag="attnT")
                nc.scalar.copy(
                    out=attnT_sb.rearrange("p c s -> p (c s)"),
                    in_=t2_ps.rearrange("p c s -> p (c s)"),
                )

                y_ps = ps_o.tile([S, D], F32, tag="ops")
                for c in range(2):
                    nc.tensor.matmul(
                        y_ps,
                        lhsT=attnT_sb[:, c, :],
                        rhs=wout_sb[:, c, :],
                        start=(c == 0), stop=(c == 1),
                    )
                y_sb = ypool.tile([S, D], F32)
                nc.vector.tensor_copy(out=y_sb, in_=y_ps)
                nc.sync.dma_start(out=y[r], in_=y_sb)

            # software-pipelined skew: rows i / i-1 / i-2 in flight so each
            # engine's static order interleaves adjacent rows' work
            state = {}
            for i in range(RPC + 2):
                if i < RPC:
                    state[i] = phase1(i)
                if 0 <= i - 1 < RPC:
                    phase2(i - 1, state[i - 1])
                if 0 <= i - 2 < RPC:
                    phase3(i - 2, state.pop(i - 2))

    nc.finalize()
    return nc


_NC = None


def _get_nc():
    global _NC
    if _NC is None:
        _NC = _build_bass()
    return _NC


def _host_prep(pair_act, pair_mask, ln_gamma, ln_beta, Wqkv, Wout):
    """Build the 8 per-core input maps (numpy only)."""
    pair_act = np.ascontiguousarray(pair_act, dtype=np.float32)
    ln_gamma = np.asarray(ln_gamma, dtype=np.float32)
    ln_beta = np.asarray(ln_beta, dtype=np.float32)
    Wqkv = np.asarray(Wqkv, dtype=np.float32)
    Wout = np.asarray(Wout, dtype=np.float32)

    # fold gamma/beta into the QKV projection (beta term is exactly zero for
    # the reference's beta=0, and the kernel does not apply a qkv bias;
    # assert so a nonzero-beta input cannot silently produce wrong results)
    W_eff = (Wqkv * ln_gamma[None, :]).T  # (256, 768): qkv = xn_z @ W_eff
    bias_eff = ln_beta @ Wqkv.T
    assert np.abs(bias_eff).max() == 0.0, "nonzero LN beta not supported"

    wqkv_h = W_eff.reshape(2, 128, 3 * D).astype(np.float16)
    wout_h = Wout.T.reshape(2, 128, D).astype(np.float16)

    # rotary tables (transposed): table[s1, c, y]
    inv_freq = 1.0 / (10000.0 ** (np.arange(0, 16, dtype=np.float32)[::2] / 16.0))
    t = np.linspace(-1.0, 1.0, S, dtype=np.float32)
    f = np.repeat(t[:, None] * inv_freq[None, :], 2, axis=-1)  # (S, 16)
    cosT = np.empty((S, ROT, S), np.float32)
    sinT = np.empty((S, ROT, S), np.float32)
    cosT[:, :16, :] = np.cos(f)[:, :, None]
    sinT[:, :16, :] = np.sin(f)[:, :, None]
    cosT[:, 16:, :] = np.cos(f).T[None, :, :]
    sinT[:, 16:, :] = np.sin(f).T[None, :, :]
    cosT = cosT.astype(np.float16)
    sinT = sinT.astype(np.float16)

    # rotate-half matrix, transposed for the PE (lhsT)
    R = np.zeros((ROT, ROT), np.float32)
    for j in range(ROT // 2):
        R[2 * j, 2 * j + 1] = -1.0
        R[2 * j + 1, 2 * j] = 1.0
    rt_h = R.T.astype(np.float16)

    x_all = pair_act.reshape(NROWS, S, D)
    maskb_all = np.where(
        np.asarray(pair_mask, bool), np.float32(MASK_BIAS), np.float32(0.0)
    ).reshape(NROWS, S)

    in_maps = []
    for core in range(N_CORES):
        r0 = core * RPC
        rows = slice(r0, r0 + RPC)
        s1 = np.arange(r0, r0 + RPC) % S
        in_maps.append({
            "x": x_all[rows],
            "cos_t": np.ascontiguousarray(cosT[s1].transpose(1, 0, 2)),
            "sin_t": np.ascontiguousarray(sinT[s1].transpose(1, 0, 2)),
            "maskb": np.ascontiguousarray(maskb_all[rows].T),  # (S, RPC)
            "wqkv": wqkv_h,
            "wout": wout_h,
            "rt": rt_h,
        })
    return in_maps


def kernel(pair_act, pair_mask, ln_gamma, ln_beta, Wqkv, Wout):
    in_maps = _host_prep(pair_act, pair_mask, ln_gamma, ln_beta, Wqkv, Wout)
    nc = _get_nc()
    res = run_bass_kernel_spmd(nc, in_maps, core_ids=list(range(N_CORES)))
    y = np.stack([res.results[i]["y"] for i in range(N_CORES)])
    return y.reshape(B, S, S, D).astype(np.float32)
